# revision 15
# baseline (speedup 1.0000x reference)
"""MDTA-style dense attention (B=2, N=4096+8 summary tokens, C=192, H=8, D=24)
on 8 Trainium2 NeuronCores.

Sharding: data-parallel over batch B (2) x tensor-parallel over heads
(4 groups of 2 heads) -> 8 cores, each core computes attention for one batch
and two heads, plus its slice of the qkv projection and the output
projection partial (Megatron row-parallel). Head partials are divided by
their softmax denominators and summed on the host during unsharding.

Device algorithm per core (all layouts transposed: feature-major):
  - qkv projection: q,k in [d, n] layout; v in [m, d] layout (V_aug with an
    appended ones column so the PV matmul also yields the softmax
    denominator). q weights are pre-scaled by temperature*log2(e) so the
    S matmul emits t = s*log2(e) (base-2 exponent units).
  - S^T blocks = k_blk^T q (contraction over d on partitions). The exp is
    computed as 2^t, split across two engines to break the ScalarE
    activation throughput wall (~1 elem/cycle/lane):
      * ScalarE units: ACTIVATE Exp with scale=ln2 (exact).
      * VectorE units: single tensor_scalar pass computing the Schraudolph
        bit-trick int16(128*t + BIAS), bit-viewed as bf16 => 2^t with ~3%
        max elementwise error that cancels in the softmax normalization
        (verified end-to-end ~9e-3 max rel err vs the 2e-2 gate).
    S-score PSUM rotates through 3 buffers (two 3-bank + one 1-bank pool)
    so both exp engines run concurrently instead of ping-ponging on a
    2-buffer recycle.
  - PV accumulation over key blocks in PSUM, column-tiled: head 0 on PE
    column strip 0, head 1 on strip 1, running concurrently. PV issue is
    software-pipelined two exp-units behind so the PE never stalls the exp
    engines on s_ps production.
  - no on-device softmax normalization: raw per-head PV sums (fp32) are
    projected with Wout (float32r matmuls) into bf16 partials, and the
    fp32 denominator rows ship separately; the host divides and reduces.
Keys are zero-padded 4104 -> 4224 (33*128); padded keys produce t=0 ->
e~1 but multiply V_aug rows that are zero (including the ones column),
so they contribute nothing.
"""

import numpy as np

import concourse.bass as bass
import concourse.tile as tile
from concourse import bacc, mybir
from concourse.bass_utils import run_bass_kernel_spmd

# Problem constants (hardcoded per contract).
B = 2
N = 4096          # output tokens
K_SUM = 8         # summary tokens
NT = N + K_SUM    # 4104 total tokens
NP = 4224         # padded key count = 33 * 128
C = 192
H = 8
D = 24
NCORES = 8

CI = 512          # query chunk (8 chunks over 4096)
MB = 128          # key block
NCHUNKS = N // CI            # 8
MBLOCKS = NP // MB           # 33

LOG2E = 1.4426950408889634
LN2 = 0.6931471805599453
TS_SCALE = 128.0
TS_BIAS = 16250.75           # Schraudolph bias (127*128 - centering)

# Exp work split per (chunk, head): ScalarE (exact exp2) takes the first
# SC_BLOCKS key blocks as 2-block units; VectorE (Schraudolph) takes the
# rest as 1-block units. Each engine has its own PSUM pool (ScalarE
# 2 bufs x 2 banks, VectorE 3 bufs x 1 bank) so the two exp pipelines
# never share buffers. Work is issued in "rounds" of 2 ScalarE units
# (head 0+1) plus 3-4 VectorE units, with PV matmuls for round r-1
# batched head-alternating so their LDWEIGHTS overlap across PE column
# strips.
SC_BLOCKS = 18            # per head; must be even
DVE_BLOCKS = MBLOCKS - SC_BLOCKS

F32 = mybir.dt.float32
F32R = mybir.dt.float32r
F16 = mybir.dt.float16
BF16 = mybir.dt.bfloat16
I16 = mybir.dt.int16

_CACHED = {}


def build_program():
    nc = bacc.Bacc("TRN2", target_bir_lowering=False, debug=False,
                   num_devices=NCORES)
    xt_d = nc.dram_tensor("XT", [C + 1, NP], F16, kind="ExternalInput")
    wt_d = nc.dram_tensor("WT", [C + 1, 160], F16, kind="ExternalInput")
    wo_d = nc.dram_tensor("WoT", [128, C], F32, kind="ExternalInput")
    out_d = nc.dram_tensor("outT", [2 * C, N], BF16, kind="ExternalOutput")
    den_d = nc.dram_tensor("den", [4, N], F32, kind="ExternalOutput")

    with tile.TileContext(nc) as tc:
        with tc.tile_pool(name="singles", bufs=1) as singles:
            xt0 = singles.tile([128, NP], F16, tag="xt0")
            xt1 = singles.tile([65, NP], F16, tag="xt1")
            wt0 = singles.tile([128, 160], F16, tag="wt0")
            wt1 = singles.tile([65, 160], F16, tag="wt1")
            wo = singles.tile([128, C], F32, tag="wo")
            wor = singles.tile([128, C], F32R, tag="wor")
            qks = singles.tile([128, NP], F16, tag="qks")
            # 4-strip replicas for row-tiled S matmuls (K=24 uses only a
            # 32-row slice of the PE array; 4 strips run concurrently)
            q4 = [singles.tile([128, N], F16, tag=f"q4_{h}", name=f"q4_{h}")
                  for h in range(2)]
            k4 = [singles.tile([128, 9 * MB], F16, tag=f"k4_{h}",
                               name=f"k4_{h}") for h in range(2)]
            vaug = singles.tile([128, MBLOCKS, 64], BF16, tag="vaug")
            # raw PV sums (f32r): 4 strips of 32 rows (head-even,
            # head-odd per head); row 0 of each strip = partial denominator
            otn = singles.tile([128, N], F32R, tag="otn")

            # Input loads (host supplies fp16). Weights first: the first
            # production matmul needs them, so they must not queue behind
            # the large XT transfers. XT is chunked so compute starts early.
            nc.sync.dma_start(out=wt0[:], in_=wt_d[0:128, :])
            nc.sync.dma_start(out=wt1[:], in_=wt_d[128:193, :])
            nc.sync.dma_start(out=wo[:], in_=wo_d[:, :])
            nc.vector.tensor_copy(out=wor[:], in_=wo[:])
            for c0 in range(0, NP, 1056):
                nc.sync.dma_start(out=xt0[:, c0:c0 + 1056],
                                  in_=xt_d[0:128, c0:c0 + 1056])
                nc.gpsimd.dma_start(out=xt1[:, c0:c0 + 1056],
                                    in_=xt_d[128:193, c0:c0 + 1056])

            xts = (xt0, xt1)
            wts = (wt0, wt1)

            # ---- q/k production: 4 roles col-tiled into one PSUM bank,
            # running concurrently on separate 32-column PE strips. Role r
            # lands at partitions 32r, matching the strip layout directly.
            with tc.tile_pool(name="qkpsum", bufs=4, space="PSUM") as qkp:
                for ci in range(9):
                    c0 = ci * CI
                    w = CI if ci < 8 else MB   # last chunk: cols 4096:4224
                    ps = qkp.tile([128, CI], F32, tag="qk")
                    for r in range(4):         # q_h0, q_h1, k_h0, k_h1
                        if ci == 8 and r < 2:
                            continue  # q only needs 4096 cols
                        for kc in range(2):
                            nc.tensor.matmul(
                                ps[32 * r:32 * r + D, :w],
                                lhsT=wts[kc][:, 24 * r:24 * r + D],
                                rhs=xts[kc][:, c0:c0 + w],
                                start=(kc == 0), stop=(kc == 1),
                                tile_position=(0, 32 * r),
                                skip_group_check=True)
                    nc.vector.tensor_copy(out=qks[:, c0:c0 + w],
                                          in_=ps[:, :w])
                    # replicate this chunk into the 4 partition strips right
                    # away so the DMAs overlap the remaining qk matmuls and
                    # the PE never idles long enough for HAM to re-throttle
                    for h in range(2):
                        q_src = qks[32 * h:32 * h + D, c0:c0 + w]
                        k_src = qks[64 + 32 * h:64 + 32 * h + D, :]
                        for st in range(4):
                            eng = nc.sync if st % 2 == 0 else nc.gpsimd
                            if ci < 8:
                                eng.dma_start(
                                    out=q4[h][32 * st:32 * st + D,
                                              c0:c0 + w],
                                    in_=q_src[:, :])
                        for mb in range(4 * ci, min(4 * (ci + 1), MBLOCKS)):
                            st, t = mb % 4, mb // 4
                            eng = nc.sync if st % 2 == 0 else nc.gpsimd
                            eng.dma_start(
                                out=k4[h][32 * st:32 * st + D,
                                          t * MB:(t + 1) * MB],
                                in_=k_src[:, mb * MB:(mb + 1) * MB])

            # ---- V_aug production: [m, d] layout, 8 key blocks batched per
            # PSUM bank so the PSUM->SBUF cast is one wide DVE op per bank.
            with tc.tile_pool(name="vpsum", bufs=2, space="PSUM") as vps:
                # V_aug per-head 32-col strip: [ones-indicator, v (24), 0*7].
                # The indicator feature row of XT makes the matmul emit the
                # ones column (and zeros for padded keys) directly.
                for mb0 in range(0, MBLOCKS, 8):
                    nb = min(8, MBLOCKS - mb0)
                    ps = vps.tile([128, 8, 64], F32, tag="v")
                    for j in range(nb):
                        m0 = (mb0 + j) * MB
                        for kc in range(2):
                            nc.tensor.matmul(
                                ps[:, j, :],
                                lhsT=xts[kc][:, m0:m0 + MB],
                                rhs=wts[kc][:, 96:160],
                                start=(kc == 0), stop=(kc == 1))
                    nc.vector.tensor_copy(out=vaug[:, mb0:mb0 + nb, :],
                                          in_=ps[:, 0:nb, :])

            # ---- attention ----
            # PSUM: sc pool 2x2 banks + dve pool 2x1 + o 2x1 bank = 8.
            with (tc.tile_pool(name="scp", bufs=2, space="PSUM") as scp,
                  tc.tile_pool(name="dvp", bufs=2, space="PSUM") as dvp,
                  tc.tile_pool(name="opsum", bufs=2, space="PSUM") as op,
                  tc.tile_pool(name="exps", bufs=12) as ep):

                def s_fill(s_ps, ci, h, m0, nblk):
                    c0 = ci * CI
                    for j in range(nblk):
                        mb = m0 + j
                        st, t = mb % 4, mb // 4
                        p0 = 32 * st
                        nc.tensor.matmul(
                            s_ps[:, j, :],
                            lhsT=k4[h][p0:p0 + D, t * MB:(t + 1) * MB],
                            rhs=q4[h][p0:p0 + D, c0:c0 + CI],
                            start=True, stop=True,
                            tile_position=(p0, 0))

                def issue_exp(eng, e_t, s_ps, sz):
                    if eng == 'D':
                        nc.vector.tensor_scalar(
                            out=e_t[:, 0:sz, :].bitcast(I16),
                            in0=s_ps[:, 0:sz, :],
                            scalar1=TS_SCALE, scalar2=TS_BIAS,
                            op0=mybir.AluOpType.mult,
                            op1=mybir.AluOpType.add)
                    else:
                        nc.scalar.activation(
                            out=e_t[:, 0:sz, :], in_=s_ps[:, 0:sz, :],
                            func=mybir.ActivationFunctionType.Exp,
                            scale=LN2)

                def issue_pv_batch(batch):
                    # batch: (o_ps, ci, h, mb, e_t, j) blocks sorted so
                    # consecutive matmuls land on different PE column strips
                    # (strip = 2*head + block parity) and co-issue; the
                    # parity split is summed for free inside the projection
                    # contraction (Wout rows duplicated).
                    for o_ps, ci, h, mb, e_t, j in sorted(
                            batch, key=lambda p: (p[3], p[2])):
                        sx = 2 * h + (mb % 2)
                        nc.tensor.matmul(
                            o_ps[32 * sx:32 * sx + 32, :],
                            lhsT=vaug[:, mb, 32 * h:32 * h + 32],
                            rhs=e_t[:, j, :],
                            start=(mb < 2), stop=(mb >= MBLOCKS - 2),
                            tile_position=(0, 32 * sx),
                            skip_group_check=True)
                        if mb == MBLOCKS - 1 and h == 1:
                            c0 = ci * CI
                            nc.vector.tensor_copy(out=otn[:, c0:c0 + CI],
                                                  in_=o_ps[:, :])

                # round-robin DVE block allocator per head
                N_ROUNDS = SC_BLOCKS // 2
                dve_per_round = [[], []]
                for h in range(2):
                    blocks = list(range(SC_BLOCKS, MBLOCKS))
                    for r in range(N_ROUNDS):
                        n = len(blocks) * (r + 1) // N_ROUNDS -                             len(blocks) * r // N_ROUNDS
                        lo = len(blocks) * r // N_ROUNDS
                        dve_per_round[h].append(blocks[lo:lo + n])

                pend = []   # PV blocks of the previous round
                for ci in range(NCHUNKS):
                    o_ps = op.tile([128, CI], F32, tag="o", name=f"o_{ci}")
                    for r in range(N_ROUNDS):
                        new_pv = []
                        for h in range(2):       # ScalarE 2-block units
                            s_ps = scp.tile([128, 2, CI], F32, tag="s",
                                            name="s_sc")
                            s_fill(s_ps, ci, h, 2 * r, 2)
                            e_t = ep.tile([128, 2, CI], BF16, tag="e")
                            issue_exp('S', e_t, s_ps, 2)
                            for j in range(2):
                                new_pv.append((o_ps, ci, h, 2 * r + j,
                                               e_t, j))
                        for h in range(2):       # VectorE 1-block units
                            for mb in dve_per_round[h][r]:
                                s_ps = dvp.tile([128, 1, CI], F32, tag="s",
                                                name="s_dv")
                                s_fill(s_ps, ci, h, mb, 1)
                                e_t = ep.tile([128, 2, CI], BF16, tag="e",
                                              name="e_d")
                                issue_exp('D', e_t, s_ps, 1)
                                new_pv.append((o_ps, ci, h, mb, e_t, 0))
                        if r in (0, 5):
                            # HAM re-warm: the steady-state loop has enough
                            # small PE gaps that the clock gate drops to
                            # K=4/8 and (at ~85% busy) never recovers on its
                            # own. A dependency-free dense burst twice per
                            # chunk forces K=8/8 back; each warm window
                            # holds for >10us of real work.
                            nburst = 24 if (ci == 0 and r == 0) else 10
                            warm = scp.tile([128, 2, CI], F32, tag="s",
                                            name="warm")
                            for i in range(nburst):
                                nc.tensor.matmul(
                                    warm[:, 0, :], lhsT=wt0[:, 0:128],
                                    rhs=xt0[:, 0:CI],
                                    start=(i == 0), stop=(i == nburst - 1))
                        # head-alternate the previous round's PV blocks so
                        # LDWEIGHTS on one column strip overlaps the matmul
                        # running on the other
                        h0b = [p for p in pend if p[2] == 0]
                        h1b = [p for p in pend if p[2] == 1]
                        inter = []
                        for i in range(max(len(h0b), len(h1b))):
                            if i < len(h0b):
                                inter.append(h0b[i])
                            if i < len(h1b):
                                inter.append(h1b[i])
                        issue_pv_batch(inter)
                        pend = new_pv
                h0b = [p for p in pend if p[2] == 0]
                h1b = [p for p in pend if p[2] == 1]
                inter = []
                for i in range(max(len(h0b), len(h1b))):
                    if i < len(h0b):
                        inter.append(h0b[i])
                    if i < len(h1b):
                        inter.append(h1b[i])
                issue_pv_batch(inter)

            # partial denominator rows (fp32) for the host-side normalize
            for sx in range(4):
                nc.sync.dma_start(out=den_d[sx:sx + 1, :],
                                  in_=otn[32 * sx:32 * sx + 1, :].bitcast(F32))

            # ---- output projection: per-head raw partials (host divides by
            # den and sums). float32r matmuls stream fp32 at full rate.
            with (tc.tile_pool(name="proja", bufs=4, space="PSUM") as pa,
                  tc.tile_pool(name="projb", bufs=4, space="PSUM") as pb,
                  tc.tile_pool(name="projsb", bufs=4) as psb):
                warm2 = pa.tile([128, CI], F32, tag="pa", name="warm2")
                for i in range(14):
                    nc.tensor.matmul(warm2[:], lhsT=wt0[:, 0:128],
                                     rhs=xt0[:, 0:CI],
                                     start=(i == 0), stop=(i == 13))
                for ci in range(NCHUNKS):
                    c0 = ci * CI
                    for h in range(2):
                        t_a = pa.tile([128, CI], F32, tag="pa")
                        t_b = pb.tile([64, CI], F32, tag="pb")
                        rhs = otn[64 * h:64 * h + 64, c0:c0 + CI]
                        nc.tensor.matmul(t_a[:],
                                         lhsT=wor[64 * h:64 * h + 64, 0:128],
                                         rhs=rhs, start=True, stop=True,
                                         tile_position=(64 * h, 0),
                                         skip_group_check=True)
                        nc.tensor.matmul(t_b[:],
                                         lhsT=wor[64 * h:64 * h + 64, 128:192],
                                         rhs=rhs, start=True, stop=True,
                                         tile_position=(64 * h, 0),
                                         skip_group_check=True)
                        s_a = psb.tile([128, CI], BF16, tag="sa")
                        s_b = psb.tile([64, CI], BF16, tag="sb")
                        nc.vector.tensor_copy(out=s_a[:], in_=t_a[:])
                        nc.scalar.copy(out=s_b[:], in_=t_b[:])
                        r0 = C * h
                        eng = nc.sync if h == 0 else nc.gpsimd
                        eng.dma_start(out=out_d[r0:r0 + 128, c0:c0 + CI],
                                      in_=s_a[:])
                        eng.dma_start(out=out_d[r0 + 128:r0 + 192, c0:c0 + CI],
                                      in_=s_b[:])

    nc.compile()
    return nc


def make_in_maps(X_flat, S_tokens, Wqkv, Wout, temperature):
    temp = np.asarray(temperature, dtype=np.float32).reshape(H)
    Wq = np.asarray(Wqkv[0:C], dtype=np.float32)
    Wk = np.asarray(Wqkv[C:2 * C], dtype=np.float32)
    Wv = np.asarray(Wqkv[2 * C:3 * C], dtype=np.float32)
    Wout = np.asarray(Wout, dtype=np.float32)

    xts = []
    for b in range(B):
        x_in = np.concatenate([np.asarray(X_flat[b], dtype=np.float32),
                               np.asarray(S_tokens[b], dtype=np.float32)], axis=0)
        xt = np.zeros((C + 1, NP), dtype=np.float32)
        xt[:C, :NT] = np.ascontiguousarray(x_in.T)
        xt[C, :NT] = 1.0  # indicator feature -> ones column of V_aug
        xts.append(xt)

    in_maps = []
    for core in range(NCORES):
        b = core // 4
        h0 = 2 * (core % 4)
        h1 = h0 + 1
        wt = np.zeros((C + 1, 160), dtype=np.float32)
        # q weights pre-scaled by temperature AND log2(e): S matmul output is
        # t = s*log2e, so exp(s) == 2^t (ScalarE applies scale=ln2 inside Exp).
        wt[:C, 0:24] = (Wq[h0 * D:(h0 + 1) * D] * (temp[h0] * LOG2E)).T
        wt[:C, 24:48] = (Wq[h1 * D:(h1 + 1) * D] * (temp[h1] * LOG2E)).T
        wt[:C, 48:72] = Wk[h0 * D:(h0 + 1) * D].T
        wt[:C, 72:96] = Wk[h1 * D:(h1 + 1) * D].T
        wt[C, 96] = 1.0                                   # ones indicator h0
        wt[:C, 97:121] = Wv[h0 * D:(h0 + 1) * D].T
        wt[C, 128] = 1.0                                  # ones indicator h1
        wt[:C, 129:153] = Wv[h1 * D:(h1 + 1) * D].T
        # per-head projection slices, duplicated across the two PV parity
        # strips so the projection contraction sums them (row 0 of each
        # 32-row strip multiplies a partial denominator row -> weight 0)
        wo = np.zeros((128, C), dtype=np.float32)
        wo[1:25] = wo[33:57] = Wout[:, h0 * D:(h0 + 1) * D].T
        wo[65:89] = wo[97:121] = Wout[:, h1 * D:(h1 + 1) * D].T
        in_maps.append({
            "XT": np.ascontiguousarray(xts[b]).astype(np.float16),
            "WT": np.ascontiguousarray(wt).astype(np.float16),
            "WoT": np.ascontiguousarray(wo).astype(np.float32),
        })
    return in_maps


def gather_output(res):
    """Host-side unshard: divide per-head raw partials by their softmax
    denominators, sum head groups, transpose to (B, N, C)."""
    out = np.zeros((B, N, C), dtype=np.float32)
    for core in range(NCORES):
        b = core // 4
        o = np.asarray(res.results[core]["outT"]).astype(np.float32)
        den = np.asarray(res.results[core]["den"]).astype(np.float32)
        out[b] += ((o[0:C] / (den[0] + den[1])[None, :]) +
                   (o[C:2 * C] / (den[2] + den[3])[None, :])).T
    return out


def run(in_maps, **kwargs):
    if "nc" not in _CACHED:
        _CACHED["nc"] = build_program()
    return run_bass_kernel_spmd(_CACHED["nc"], in_maps,
                                core_ids=list(range(NCORES)), **kwargs)


def kernel(X_flat, S_tokens, Wqkv, Wout, temperature):
    in_maps = make_in_maps(X_flat, S_tokens, Wqkv, Wout, temperature)
    res = run(in_maps)
    return gather_output(res)
